# revision 8
# baseline (speedup 1.0000x reference)
"""Trainium2 Bass kernel for nn_Attention_13537736917778.

Full inputs -> full output. Sharding: 8 cores = 2 (batch) x 4 (head groups of 4).
Per-core: channel-major flash attention (S^T layout, keys on partitions).
Softmax denominators are produced replicated across 64 PSUM rows by ones-columns
in the PV stationary operand; normalization and RMS-norm reciprocals both run as
ACT ln/exp pairs (exp table set only), so no DVE reciprocal and no PE broadcast
matmuls. Out-projection partial sums are reduced on host.
"""
import sys
import numpy as np

sys.path.insert(0, "/opt/trn_rl_repo")

import ml_dtypes
import concourse.bass as bass
import concourse.mybir as mybir
from concourse import tile
from concourse.bass_utils import run_bass_kernel_spmd
from contextlib import ExitStack

bf16 = mybir.dt.bfloat16
f32 = mybir.dt.float32

B, N, C = 2, 2048, 1024
H, D = 16, 64
G = 4              # heads per core
NT = N             # tokens per core (one batch)
FT = 512
TI = NT // FT      # 4 i-tiles
KC = C // 128      # 8 input-channel chunks
JC = NT // 128     # 16 key chunks
OC = 3 * G * D // 128   # 6 qkv output chunks
EPS = 1e-6
SCALE = 1.0 / 8.0  # 1/sqrt(64)

_CACHE = {}


def _split_waits(nc, limit=1):
    """walrus CTRL has one hw wait slot; split multi-wait instructions into
    NOP chains carrying the extra waits."""
    counter = 0
    for fn in nc.m.functions:
        for bb in fn.blocks:
            new_insts = []
            for inst in bb.instructions:
                si = inst.sync_info
                if si is not None and si.on_wait and len(si.on_wait) > limit:
                    waits = list(si.on_wait)
                    head, tail = waits[:-limit], waits[-limit:]
                    for i in range(0, len(head), limit):
                        nop = mybir.InstNoOp(
                            name=f"I-waitsplit-{counter}", ins=[], outs=[]
                        )
                        counter += 1
                        nop.engine = inst.engine
                        nop.sync_info = mybir.SyncInfo(
                            on_wait=head[i : i + limit], on_update=[]
                        )
                        new_insts.append(nop)
                    inst.sync_info = mybir.SyncInfo(
                        on_wait=tail, on_update=list(si.on_update)
                    )
                new_insts.append(inst)
            bb.instructions[:] = new_insts
    return counter


def _build_nc():
    nc = bass.Bass()
    xT = nc.declare_dram_parameter("xT", [C, NT], bf16, isOutput=False)
    wqkvT = nc.declare_dram_parameter("wqkvT", [C, OC * 128], bf16, isOutput=False)
    bqkv = nc.declare_dram_parameter("bqkv", [128, OC], f32, isOutput=False)
    wrep = nc.declare_dram_parameter("wrep", [128, 2], f32, isOutput=False)
    iden = nc.declare_dram_parameter("iden", [128, 128], bf16, isOutput=False)
    woT = nc.declare_dram_parameter("woT", [2, 128, C], bf16, isOutput=False)
    y = nc.declare_dram_parameter("y", [NT, C], f32, isOutput=True)

    xT_r = xT.rearrange("(kc p) n -> kc p n", p=128)
    wqkvT_r = wqkvT.rearrange("(kc p) o -> kc p o", p=128)

    Exp = mybir.ActivationFunctionType.Exp
    Ln = mybir.ActivationFunctionType.Ln
    MUL = mybir.AluOpType.mult

    with tile.TileContext(nc) as tc:
        with ExitStack() as ctx:
            perm = ctx.enter_context(tc.tile_pool(name="perm", bufs=1))
            # ---- persistent tiles ----
            iden_sb = perm.tile([128, 128], bf16, name="iden_sb", tag="iden_sb")
            nc.sync.dma_start(out=iden_sb[:], in_=iden[:])
            bias_sb = perm.tile([128, OC + 1], f32, name="bias_sb", tag="bias_sb")
            nc.sync.dma_start(out=bias_sb[:, 0:OC], in_=bqkv[:])
            nc.vector.memset(bias_sb[:, OC : OC + 1], EPS)
            wrep_sb = perm.tile([128, 2], f32, name="wrep_sb", tag="wrep_sb")
            nc.sync.dma_start(out=wrep_sb[:], in_=wrep[:])
            ones_mask = perm.tile([128, 128], bf16, name="ones_mask", tag="ones_mask")
            nc.vector.memset(ones_mask[:], 0.0)
            nc.vector.memset(ones_mask[0:64, 0:64], 1.0)
            nc.vector.memset(ones_mask[64:128, 64:128], 1.0)

            w_sb = []
            for kc in range(KC):
                t = perm.tile([128, OC * 128], bf16, name=f"w_sb{kc}", tag=f"w_sb{kc}")
                nc.sync.dma_start(out=t[:], in_=wqkvT_r[kc])
                w_sb.append(t)
            woT_sb = []
            for oo in range(2):
                t = perm.tile([128, C], bf16, name=f"woT_sb{oo}", tag=f"woT_sb{oo}")
                nc.sync.dma_start(out=t[:], in_=woT[oo])
                woT_sb.append(t)

            qk_sb = [
                perm.tile([128, NT], f32, name=f"qk_sb{i}", tag=f"qk_sb{i}")
                for i in range(4)
            ]
            v16_sb = [
                perm.tile([128, NT], bf16, name=f"v16_sb{i}", tag=f"v16_sb{i}")
                for i in range(2)
            ]
            qhat = [
                perm.tile([128, NT], bf16, name=f"qhat{i}", tag=f"qhat{i}")
                for i in range(2)
            ]
            khat = [
                perm.tile([128, NT], bf16, name=f"khat{i}", tag=f"khat{i}")
                for i in range(2)
            ]
            # per jc: [A-V(64) | A-ones(64) | B-V(64) | B-ones(64)]
            vtok = [
                perm.tile([128, JC, 256], bf16, name=f"vtok{i}", tag=f"vtok{i}")
                for i in range(2)
            ]
            onT = [
                perm.tile([128, NT], bf16, name=f"onT{i}", tag=f"onT{i}")
                for i in range(2)
            ]
            for c in range(2):
                nc.vector.memset(vtok[c][:, :, 64:128], 1.0)
                nc.vector.memset(vtok[c][:, :, 192:256], 1.0)

            # ---- pools (whole kernel) ----
            xpool = ctx.enter_context(tc.tile_pool(name="xpool", bufs=34))
            sqpool = ctx.enter_context(tc.tile_pool(name="sqpool", bufs=2))
            t1pool = ctx.enter_context(tc.tile_pool(name="t1pool", bufs=1))
            rrpool = ctx.enter_context(tc.tile_pool(name="rrpool", bufs=2))
            ptpool = ctx.enter_context(tc.tile_pool(name="ptpool", bufs=3))
            dnpool = ctx.enter_context(tc.tile_pool(name="dnpool", bufs=1))
            ypool = ctx.enter_context(tc.tile_pool(name="ypool", bufs=2))
            # PSUM: tag "s4" = 2 slots x [128,1536] (6 banks), tag "pv" = 1 slot x
            # [128,1024] (2 banks)
            spool = ctx.enter_context(tc.tile_pool(name="spool", bufs=2, space="PSUM"))
            pvpool = ctx.enter_context(
                tc.tile_pool(name="pvpool", bufs=1, space="PSUM")
            )

            xt = {}
            cnt = [0]

            def emit_x_dma(it):
                isl = slice(it * FT, (it + 1) * FT)
                for kc in range(KC):
                    t = xpool.tile([128, FT], bf16, name=f"xt{it}_{kc}", tag="xt")
                    nc.sync.dma_start(out=t[:], in_=xT_r[kc][:, isl])
                    xt[(it, kc)] = t

            def emit_qkv_group(it, oc):
                isl = slice(it * FT, (it + 1) * FT)
                cnt[0] += 1
                ps = spool.tile(
                    [128, 3 * FT], f32, name=f"qp{cnt[0]}", tag="s4"
                )
                for kc in range(KC):
                    nc.tensor.matmul(
                        ps[:, 0:FT],
                        lhsT=w_sb[kc][:, oc * 128 : (oc + 1) * 128],
                        rhs=xt[(it, kc)][:],
                        start=(kc == 0),
                        stop=(kc == KC - 1),
                    )
                dst = qk_sb[oc] if oc < 4 else v16_sb[oc - 4]
                nc.vector.tensor_scalar_add(
                    dst[:, isl], ps[:, 0:FT], bias_sb[:, oc : oc + 1]
                )

            def emit_vtrans(c, jc):
                cnt[0] += 1
                pt = spool.tile([128, 128], bf16, name=f"vt{cnt[0]}", tag="s4")
                nc.tensor.transpose(
                    pt[:], v16_sb[c][:, jc * 128 : (jc + 1) * 128], iden_sb[:]
                )
                nc.vector.tensor_copy(vtok[c][:, jc, 0:64], pt[:, 0:64])
                nc.vector.tensor_copy(vtok[c][:, jc, 128:192], pt[:, 64:128])

            def emit_rms(qc, qk_i):
                src_t = qk_sb[qc] if qk_i == 0 else qk_sb[2 + qc]
                dst = qhat[qc] if qk_i == 0 else khat[qc]
                sq = sqpool.tile([128, NT], bf16, name=f"sq{qc}_{qk_i}", tag="sq")
                nc.vector.tensor_mul(sq[:], src_t[:], src_t[:])
                rr = rrpool.tile([128, NT], f32, name=f"rr{qc}_{qk_i}", tag="rr")
                for half in range(2):
                    hs = slice(half * 2 * FT, (half + 1) * 2 * FT)
                    cnt[0] += 1
                    ms = pvpool.tile(
                        [128, 2 * FT], f32, name=f"ms{cnt[0]}", tag="pv"
                    )
                    for t in range(2):
                        tsl = slice(t * FT, (t + 1) * FT)
                        gsl = slice((half * 2 + t) * FT, (half * 2 + t + 1) * FT)
                        nc.tensor.matmul(
                            ms[:, tsl],
                            lhsT=ones_mask[:],
                            rhs=sq[:, gsl],
                            start=True,
                            stop=True,
                        )
                    t1 = t1pool.tile(
                        [128, 2 * FT], f32, name=f"t1_{qc}_{qk_i}_{half}", tag="t1"
                    )
                    nc.scalar.activation(
                        t1[:], ms[:], Ln, scale=1.0 / D, bias=bias_sb[:, OC : OC + 1]
                    )
                    nc.scalar.activation(rr[:, hs], t1[:], Exp, scale=-0.5)
                nc.vector.scalar_tensor_tensor(
                    dst[:], src_t[:], wrep_sb[:, qk_i : qk_i + 1], rr[:], MUL, MUL
                )

            def emit_attention(qc, it, sprinkle=None):
                # sprinkle: dict s -> list of thunks to emit after step s
                isl = slice(it * FT, (it + 1) * FT)
                pv2 = pvpool.tile(
                    [128, 2 * FT], f32, name=f"pv2_{qc}_{it}", tag="pv"
                )
                NSEQ = 2 * JC
                PER = 3
                s3 = None
                for s in range(NSEQ):
                    head = s % 2
                    jc = s // 2
                    jsl = slice(jc * 128, (jc + 1) * 128)
                    slot = s % PER
                    if slot == 0:
                        s3 = spool.tile(
                            [128, PER * FT], f32, name=f"s3_{qc}_{it}_{s}", tag="s4"
                        )
                    ssl = slice(slot * FT, (slot + 1) * FT)
                    hsl = slice(head * 64, (head + 1) * 64)
                    nc.tensor.matmul(
                        s3[:, ssl],
                        lhsT=khat[qc][hsl, jsl],
                        rhs=qhat[qc][hsl, isl],
                        start=True,
                        stop=True,
                    )
                    if slot == PER - 1 or s == NSEQ - 1:
                        nfull = slot + 1
                        pt3 = ptpool.tile(
                            [128, PER * FT], bf16, name=f"pt{qc}_{it}_{s}", tag="pt"
                        )
                        nc.scalar.activation(
                            pt3[:, 0 : nfull * FT],
                            s3[:, 0 : nfull * FT],
                            Exp,
                            scale=SCALE,
                        )
                        for sb in range(s - nfull + 1, s + 1):
                            hb = sb % 2
                            jb = sb // 2
                            bsl = slice((sb % PER) * FT, (sb % PER + 1) * FT)
                            nc.tensor.matmul(
                                pv2[:, hb * FT : (hb + 1) * FT],
                                lhsT=vtok[qc][:, jb, hb * 128 : (hb + 1) * 128],
                                rhs=pt3[:, bsl],
                                start=(sb == hb),
                                stop=(sb >= NSEQ - 2),
                            )
                    if sprinkle and s in sprinkle:
                        for thunk in sprinkle[s]:
                            thunk()
                # normalize: O = PV * exp(-ln(denom)); denom replicated rows 64:128
                td = dnpool.tile([64, 2 * FT], f32, name=f"td{qc}_{it}", tag="td")
                nc.scalar.activation(td[:], pv2[64:128, :], Ln)
                bcr = dnpool.tile([64, 2 * FT], f32, name=f"bcr{qc}_{it}", tag="bcr")
                nc.scalar.activation(bcr[:], td[:], Exp, scale=-1.0)
                nc.vector.tensor_mul(onT[qc][0:64, isl], pv2[0:64, 0:FT], bcr[:, 0:FT])
                nc.vector.tensor_mul(
                    onT[qc][64:128, isl], pv2[0:64, FT : 2 * FT], bcr[:, FT : 2 * FT]
                )

            def emit_outproj(ic, tagname):
                csl = slice(ic * 128, (ic + 1) * 128)
                shape = [128, 3 * FT] if tagname == "s4" else [128, 2 * FT]
                pool = spool if tagname == "s4" else pvpool
                p01 = pool.tile(shape, f32, name=f"p01_{ic}", tag=tagname)
                for oo in range(2):
                    nc.tensor.matmul(
                        p01[:, 0:FT],
                        lhsT=onT[oo][:, csl],
                        rhs=woT_sb[oo][:, 0:FT],
                        start=(oo == 0),
                        stop=(oo == 1),
                    )
                    nc.tensor.matmul(
                        p01[:, FT : 2 * FT],
                        lhsT=onT[oo][:, csl],
                        rhs=woT_sb[oo][:, FT : 2 * FT],
                        start=(oo == 0),
                        stop=(oo == 1),
                    )
                yt = ypool.tile([128, C], f32, name=f"yt{ic}", tag="yt")
                nc.vector.tensor_copy(yt[:], p01[:, 0 : 2 * FT])
                nc.sync.dma_start(out=y[csl, :], in_=yt[:])

            # ---------------- emission schedule ----------------
            for it in range(TI):
                emit_x_dma(it)
            # phase A prefix: head-pair 0 projections (q0, k0, v0)
            for oc in (0, 2):
                for it in range(TI):
                    emit_qkv_group(it, oc)
            for it in range(TI):
                emit_qkv_group(it, 4)
                for jc in range(it * 4, it * 4 + 4):
                    emit_vtrans(0, jc)
            emit_rms(0, 0)
            emit_rms(0, 1)
            # attention for pair 0, with pair-1 projections sprinkled in
            for it in range(TI):
                spr = {
                    9: [lambda it=it: emit_qkv_group(it, 1)],
                    19: [lambda it=it: emit_qkv_group(it, 3)],
                    29: [lambda it=it: emit_qkv_group(it, 5)]
                    + (
                        [
                            lambda it=it: [
                                emit_vtrans(1, jc)
                                for jc in range((it - 1) * 4, it * 4)
                            ]
                        ]
                        if it > 0
                        else []
                    ),
                }
                emit_attention(0, it, sprinkle=spr)
            for jc in range(12, 16):
                emit_vtrans(1, jc)
            emit_rms(1, 0)
            emit_rms(1, 1)
            for it in range(TI):
                emit_attention(1, it)
            for ic in range(16):
                emit_outproj(ic, ("s4", "s4", "pv")[ic % 3])

    _split_waits(nc, limit=1)
    return nc


def _prep_inputs(x, Wq, bq, Wk, bk, Wv, bv, q_norm_w, k_norm_w, Wo, bo):
    bf = ml_dtypes.bfloat16
    x = np.asarray(x, dtype=np.float32)
    Wfull = np.concatenate(
        [np.asarray(Wq), np.asarray(Wk), np.asarray(Wv)], axis=0
    ).astype(np.float32)
    bfull = np.concatenate(
        [np.asarray(bq), np.asarray(bk), np.asarray(bv)], axis=0
    ).astype(np.float32)
    Wo = np.asarray(Wo, dtype=np.float32)
    q_norm_w = np.asarray(q_norm_w, dtype=np.float32)
    k_norm_w = np.asarray(k_norm_w, dtype=np.float32)

    xT_b = [np.ascontiguousarray(x[b].T).astype(bf) for b in range(B)]
    IDEN = np.eye(128, dtype=np.float32).astype(bf)
    wrep = np.stack(
        [np.tile(q_norm_w, 2), np.tile(k_norm_w, 2)], axis=1
    ).astype(np.float32)

    in_maps = []
    for core in range(8):
        b = core // 4
        hg = core % 4
        heads = [hg * 4 + i for i in range(G)]
        q_rows = np.concatenate([Wfull[192 * h : 192 * h + 64] for h in heads], axis=0)
        k_rows = np.concatenate(
            [Wfull[192 * h + 64 : 192 * h + 128] for h in heads], axis=0
        )
        v_rows = np.concatenate(
            [Wfull[192 * h + 128 : 192 * h + 192] for h in heads], axis=0
        )
        W_shard = np.concatenate([q_rows, k_rows, v_rows], axis=0)  # [768, 1024]
        bq_rows = np.concatenate([bfull[192 * h : 192 * h + 64] for h in heads])
        bk_rows = np.concatenate([bfull[192 * h + 64 : 192 * h + 128] for h in heads])
        bv_rows = np.concatenate([bfull[192 * h + 128 : 192 * h + 192] for h in heads])
        b_shard = np.concatenate([bq_rows, bk_rows, bv_rows])  # [768]
        cols = np.concatenate([np.arange(64 * h, 64 * h + 64) for h in heads])
        WoT_shard = np.ascontiguousarray(Wo[:, cols].T)  # [256, 1024]

        in_maps.append(
            {
                "xT": xT_b[b],
                "wqkvT": np.ascontiguousarray(W_shard.T).astype(bf),
                "bqkv": np.ascontiguousarray(b_shard.reshape(OC, 128).T).astype(
                    np.float32
                ),
                "wrep": wrep,
                "iden": IDEN,
                "woT": WoT_shard.reshape(2, 128, C).astype(bf),
            }
        )
    return in_maps


def kernel(**inputs):
    if "nc" not in _CACHE:
        _CACHE["nc"] = _build_nc()
    nc = _CACHE["nc"]
    in_maps = _prep_inputs(**inputs)
    res = run_bass_kernel_spmd(nc, in_maps, list(range(8)))
    bo = np.asarray(inputs["bo"], dtype=np.float32)
    y = np.zeros((B, N, C), dtype=np.float32)
    for core in range(8):
        y[core // 4] += res.results[core]["y"]
    y += bo[None, None, :]
    return y


# revision 9
# speedup vs baseline: 1.0028x; 1.0028x over previous
"""Trainium2 Bass kernel for nn_Attention_13537736917778.

Full inputs -> full output. Sharding: 8 cores = 2 (batch) x 4 (head groups of 4).
Per-core: channel-major flash attention (S^T layout, keys on partitions).
Softmax denominators are produced replicated across 64 PSUM rows by ones-columns
in the PV stationary operand; normalization and RMS-norm reciprocals both run as
ACT ln/exp pairs (exp table set only), so no DVE reciprocal and no PE broadcast
matmuls. Out-projection partial sums are reduced on host.
"""
import sys
import numpy as np

sys.path.insert(0, "/opt/trn_rl_repo")

import ml_dtypes
import concourse.bass as bass
import concourse.mybir as mybir
from concourse import tile
from concourse.bass_utils import run_bass_kernel_spmd
from contextlib import ExitStack

bf16 = mybir.dt.bfloat16
f32 = mybir.dt.float32

B, N, C = 2, 2048, 1024
H, D = 16, 64
G = 4              # heads per core
NT = N             # tokens per core (one batch)
FT = 512
TI = NT // FT      # 4 i-tiles
KC = C // 128      # 8 input-channel chunks
JC = NT // 128     # 16 key chunks
OC = 3 * G * D // 128   # 6 qkv output chunks
EPS = 1e-6
SCALE = 1.0 / 8.0  # 1/sqrt(64)

_CACHE = {}


def _split_waits(nc, limit=1):
    """walrus CTRL has one hw wait slot; split multi-wait instructions into
    NOP chains carrying the extra waits."""
    counter = 0
    for fn in nc.m.functions:
        for bb in fn.blocks:
            new_insts = []
            for inst in bb.instructions:
                si = inst.sync_info
                if si is not None and si.on_wait and len(si.on_wait) > limit:
                    waits = list(si.on_wait)
                    head, tail = waits[:-limit], waits[-limit:]
                    for i in range(0, len(head), limit):
                        nop = mybir.InstNoOp(
                            name=f"I-waitsplit-{counter}", ins=[], outs=[]
                        )
                        counter += 1
                        nop.engine = inst.engine
                        nop.sync_info = mybir.SyncInfo(
                            on_wait=head[i : i + limit], on_update=[]
                        )
                        new_insts.append(nop)
                    inst.sync_info = mybir.SyncInfo(
                        on_wait=tail, on_update=list(si.on_update)
                    )
                new_insts.append(inst)
            bb.instructions[:] = new_insts
    return counter


def _build_nc():
    nc = bass.Bass()
    xT = nc.declare_dram_parameter("xT", [C, NT], bf16, isOutput=False)
    wqkvT = nc.declare_dram_parameter("wqkvT", [C, OC * 128], bf16, isOutput=False)
    bqkv = nc.declare_dram_parameter("bqkv", [128, OC], f32, isOutput=False)
    wrep = nc.declare_dram_parameter("wrep", [128, 2], f32, isOutput=False)
    iden = nc.declare_dram_parameter("iden", [128, 128], bf16, isOutput=False)
    woT = nc.declare_dram_parameter("woT", [2, 128, C], bf16, isOutput=False)
    y = nc.declare_dram_parameter("y", [NT, C], f32, isOutput=True)

    xT_r = xT.rearrange("(kc p) n -> kc p n", p=128)
    wqkvT_r = wqkvT.rearrange("(kc p) o -> kc p o", p=128)

    Exp = mybir.ActivationFunctionType.Exp
    Ln = mybir.ActivationFunctionType.Ln
    MUL = mybir.AluOpType.mult

    with tile.TileContext(nc) as tc:
        with ExitStack() as ctx:
            perm = ctx.enter_context(tc.tile_pool(name="perm", bufs=1))
            # ---- persistent tiles ----
            iden_sb = perm.tile([128, 128], bf16, name="iden_sb", tag="iden_sb")
            nc.sync.dma_start(out=iden_sb[:], in_=iden[:])
            bias_sb = perm.tile([128, OC + 1], f32, name="bias_sb", tag="bias_sb")
            nc.sync.dma_start(out=bias_sb[:, 0:OC], in_=bqkv[:])
            nc.vector.memset(bias_sb[:, OC : OC + 1], EPS)
            wrep_sb = perm.tile([128, 2], f32, name="wrep_sb", tag="wrep_sb")
            nc.sync.dma_start(out=wrep_sb[:], in_=wrep[:])
            ones_mask = perm.tile([128, 128], bf16, name="ones_mask", tag="ones_mask")
            nc.vector.memset(ones_mask[:], 0.0)
            nc.vector.memset(ones_mask[0:64, 0:64], 1.0)
            nc.vector.memset(ones_mask[64:128, 64:128], 1.0)

            w_sb = []
            for kc in range(KC):
                t = perm.tile([128, OC * 128], bf16, name=f"w_sb{kc}", tag=f"w_sb{kc}")
                nc.sync.dma_start(out=t[:], in_=wqkvT_r[kc])
                w_sb.append(t)
            woT_sb = []
            for oo in range(2):
                t = perm.tile([128, C], bf16, name=f"woT_sb{oo}", tag=f"woT_sb{oo}")
                nc.sync.dma_start(out=t[:], in_=woT[oo])
                woT_sb.append(t)

            qk_sb = [
                perm.tile([128, NT], f32, name=f"qk_sb{i}", tag=f"qk_sb{i}")
                for i in range(4)
            ]
            v16_sb = [
                perm.tile([128, NT], bf16, name=f"v16_sb{i}", tag=f"v16_sb{i}")
                for i in range(2)
            ]
            qhat = [
                perm.tile([128, NT], bf16, name=f"qhat{i}", tag=f"qhat{i}")
                for i in range(2)
            ]
            khat = [
                perm.tile([128, NT], bf16, name=f"khat{i}", tag=f"khat{i}")
                for i in range(2)
            ]
            # per jc: [A-V(64) | A-ones(64) | B-V(64) | B-ones(64)]
            vtok = [
                perm.tile([128, JC, 256], bf16, name=f"vtok{i}", tag=f"vtok{i}")
                for i in range(2)
            ]
            onT = [
                perm.tile([128, NT], bf16, name=f"onT{i}", tag=f"onT{i}")
                for i in range(2)
            ]
            for c in range(2):
                nc.vector.memset(vtok[c][:, :, 64:128], 1.0)
                nc.vector.memset(vtok[c][:, :, 192:256], 1.0)

            # ---- pools (whole kernel) ----
            xpool = ctx.enter_context(tc.tile_pool(name="xpool", bufs=34))
            sqpool = ctx.enter_context(tc.tile_pool(name="sqpool", bufs=2))
            t1pool = ctx.enter_context(tc.tile_pool(name="t1pool", bufs=1))
            rrpool = ctx.enter_context(tc.tile_pool(name="rrpool", bufs=2))
            ptpool = ctx.enter_context(tc.tile_pool(name="ptpool", bufs=3))
            dnpool = ctx.enter_context(tc.tile_pool(name="dnpool", bufs=1))
            ypool = ctx.enter_context(tc.tile_pool(name="ypool", bufs=2))
            # PSUM: tag "s4" = 2 slots x [128,1536] (6 banks), tag "pv" = 1 slot x
            # [128,1024] (2 banks)
            spool = ctx.enter_context(tc.tile_pool(name="spool", bufs=2, space="PSUM"))
            pvpool = ctx.enter_context(
                tc.tile_pool(name="pvpool", bufs=1, space="PSUM")
            )

            xt = {}
            cnt = [0]

            def emit_x_dma(it):
                isl = slice(it * FT, (it + 1) * FT)
                for kc in range(KC):
                    t = xpool.tile([128, FT], bf16, name=f"xt{it}_{kc}", tag="xt")
                    nc.sync.dma_start(out=t[:], in_=xT_r[kc][:, isl])
                    xt[(it, kc)] = t

            def emit_qkv_group(it, oc):
                isl = slice(it * FT, (it + 1) * FT)
                cnt[0] += 1
                ps = spool.tile(
                    [128, 3 * FT], f32, name=f"qp{cnt[0]}", tag="s4"
                )
                for kc in range(KC):
                    nc.tensor.matmul(
                        ps[:, 0:FT],
                        lhsT=w_sb[kc][:, oc * 128 : (oc + 1) * 128],
                        rhs=xt[(it, kc)][:],
                        start=(kc == 0),
                        stop=(kc == KC - 1),
                    )
                dst = qk_sb[oc] if oc < 4 else v16_sb[oc - 4]
                nc.vector.tensor_scalar_add(
                    dst[:, isl], ps[:, 0:FT], bias_sb[:, oc : oc + 1]
                )

            def emit_vtrans(c, jc):
                cnt[0] += 1
                pt = spool.tile([128, 128], bf16, name=f"vt{cnt[0]}", tag="s4")
                nc.tensor.transpose(
                    pt[:], v16_sb[c][:, jc * 128 : (jc + 1) * 128], iden_sb[:]
                )
                nc.vector.tensor_copy(vtok[c][:, jc, 0:64], pt[:, 0:64])
                nc.vector.tensor_copy(vtok[c][:, jc, 128:192], pt[:, 64:128])

            def emit_rms(qc, qk_i):
                src_t = qk_sb[qc] if qk_i == 0 else qk_sb[2 + qc]
                dst = qhat[qc] if qk_i == 0 else khat[qc]
                sq = sqpool.tile([128, NT], bf16, name=f"sq{qc}_{qk_i}", tag="sq")
                nc.vector.tensor_mul(sq[:], src_t[:], src_t[:])
                rr = rrpool.tile([128, NT], f32, name=f"rr{qc}_{qk_i}", tag="rr")
                for half in range(2):
                    hs = slice(half * 2 * FT, (half + 1) * 2 * FT)
                    cnt[0] += 1
                    ms = pvpool.tile(
                        [128, 2 * FT], f32, name=f"ms{cnt[0]}", tag="pv"
                    )
                    for t in range(2):
                        tsl = slice(t * FT, (t + 1) * FT)
                        gsl = slice((half * 2 + t) * FT, (half * 2 + t + 1) * FT)
                        nc.tensor.matmul(
                            ms[:, tsl],
                            lhsT=ones_mask[:],
                            rhs=sq[:, gsl],
                            start=True,
                            stop=True,
                        )
                    t1 = t1pool.tile(
                        [128, 2 * FT], f32, name=f"t1_{qc}_{qk_i}_{half}", tag="t1"
                    )
                    nc.scalar.activation(
                        t1[:], ms[:], Ln, scale=1.0 / D, bias=bias_sb[:, OC : OC + 1]
                    )
                    nc.scalar.activation(rr[:, hs], t1[:], Exp, scale=-0.5)
                nc.vector.scalar_tensor_tensor(
                    dst[:], src_t[:], wrep_sb[:, qk_i : qk_i + 1], rr[:], MUL, MUL
                )

            def emit_attention(qc, it, sprinkle=None):
                # sprinkle: dict s -> list of thunks to emit after step s
                isl = slice(it * FT, (it + 1) * FT)
                pv2 = pvpool.tile(
                    [128, 2 * FT], f32, name=f"pv2_{qc}_{it}", tag="pv"
                )
                NSEQ = 2 * JC
                PER = 3
                s3 = None
                for s in range(NSEQ):
                    head = s % 2
                    jc = s // 2
                    jsl = slice(jc * 128, (jc + 1) * 128)
                    slot = s % PER
                    if slot == 0:
                        s3 = spool.tile(
                            [128, PER * FT], f32, name=f"s3_{qc}_{it}_{s}", tag="s4"
                        )
                    ssl = slice(slot * FT, (slot + 1) * FT)
                    hsl = slice(head * 64, (head + 1) * 64)
                    nc.tensor.matmul(
                        s3[:, ssl],
                        lhsT=khat[qc][hsl, jsl],
                        rhs=qhat[qc][hsl, isl],
                        start=True,
                        stop=True,
                    )
                    if slot == PER - 1 or s == NSEQ - 1:
                        nfull = slot + 1
                        pt3 = ptpool.tile(
                            [128, PER * FT], bf16, name=f"pt{qc}_{it}_{s}", tag="pt"
                        )
                        nc.scalar.activation(
                            pt3[:, 0 : nfull * FT],
                            s3[:, 0 : nfull * FT],
                            Exp,
                            scale=SCALE,
                        )
                        for sb in range(s - nfull + 1, s + 1):
                            hb = sb % 2
                            jb = sb // 2
                            bsl = slice((sb % PER) * FT, (sb % PER + 1) * FT)
                            nc.tensor.matmul(
                                pv2[:, hb * FT : (hb + 1) * FT],
                                lhsT=vtok[qc][:, jb, hb * 128 : (hb + 1) * 128],
                                rhs=pt3[:, bsl],
                                start=(sb == hb),
                                stop=(sb >= NSEQ - 2),
                            )
                    if sprinkle and s in sprinkle:
                        for thunk in sprinkle[s]:
                            thunk()
                # normalize: O = PV * exp(-ln(denom)); denom replicated rows 64:128
                td = dnpool.tile([64, 2 * FT], f32, name=f"td{qc}_{it}", tag="td")
                nc.scalar.activation(td[:], pv2[64:128, :], Ln)
                bcr = dnpool.tile([64, 2 * FT], f32, name=f"bcr{qc}_{it}", tag="bcr")
                nc.scalar.activation(bcr[:], td[:], Exp, scale=-1.0)
                nc.vector.tensor_mul(onT[qc][0:64, isl], pv2[0:64, 0:FT], bcr[:, 0:FT])
                nc.vector.tensor_mul(
                    onT[qc][64:128, isl], pv2[0:64, FT : 2 * FT], bcr[:, FT : 2 * FT]
                )

            def emit_outproj(ic, tagname):
                csl = slice(ic * 128, (ic + 1) * 128)
                shape = [128, 3 * FT] if tagname == "s4" else [128, 2 * FT]
                pool = spool if tagname == "s4" else pvpool
                p01 = pool.tile(shape, f32, name=f"p01_{ic}", tag=tagname)
                for oo in range(2):
                    nc.tensor.matmul(
                        p01[:, 0:FT],
                        lhsT=onT[oo][:, csl],
                        rhs=woT_sb[oo][:, 0:FT],
                        start=(oo == 0),
                        stop=(oo == 1),
                    )
                    nc.tensor.matmul(
                        p01[:, FT : 2 * FT],
                        lhsT=onT[oo][:, csl],
                        rhs=woT_sb[oo][:, FT : 2 * FT],
                        start=(oo == 0),
                        stop=(oo == 1),
                    )
                yt = ypool.tile([128, C], f32, name=f"yt{ic}", tag="yt")
                nc.vector.tensor_copy(yt[:], p01[:, 0 : 2 * FT])
                nc.sync.dma_start(out=y[csl, :], in_=yt[:])

            # ---------------- emission schedule ----------------
            for it in range(TI):
                emit_x_dma(it)
            # pair-0 projections with early rms so ACT starts ASAP
            for it in range(TI):
                emit_qkv_group(it, 0)
            emit_rms(0, 0)
            for it in range(TI):
                emit_qkv_group(it, 2)
            emit_rms(0, 1)
            for it in range(TI):
                emit_qkv_group(it, 4)
                for jc in range(it * 4, it * 4 + 4):
                    emit_vtrans(0, jc)
            # attention for pair 0, with pair-1 projections sprinkled in
            for it in range(TI):
                spr = {
                    7: [lambda it=it: emit_qkv_group(it, 1)],
                    15: [lambda it=it: emit_qkv_group(it, 3)],
                    23: [lambda it=it: emit_qkv_group(it, 5)]
                    + (
                        [
                            lambda it=it: [
                                emit_vtrans(1, jc)
                                for jc in range((it - 1) * 4, it * 4)
                            ]
                        ]
                        if it > 0
                        else []
                    ),
                }
                emit_attention(0, it, sprinkle=spr)
            for jc in range(12, 16):
                emit_vtrans(1, jc)
            emit_rms(1, 0)
            emit_rms(1, 1)
            # attention for pair 1, with out-projection of prior i-tiles sprinkled in
            for it in range(TI):
                if it == 0:
                    spr = None
                else:
                    base = (it - 1) * 4
                    spr = {
                        7: [lambda ic=base: emit_outproj(ic, "s4")],
                        15: [lambda ic=base + 1: emit_outproj(ic, "s4")],
                        23: [
                            lambda ic=base + 2: emit_outproj(ic, "s4"),
                            lambda ic=base + 3: emit_outproj(ic, "s4"),
                        ],
                    }
                emit_attention(1, it, sprinkle=spr)
            for ic in range(12, 16):
                emit_outproj(ic, ("s4", "s4", "pv", "s4")[ic % 4])

    _split_waits(nc, limit=1)
    return nc


def _prep_inputs(x, Wq, bq, Wk, bk, Wv, bv, q_norm_w, k_norm_w, Wo, bo):
    bf = ml_dtypes.bfloat16
    x = np.asarray(x, dtype=np.float32)
    Wfull = np.concatenate(
        [np.asarray(Wq), np.asarray(Wk), np.asarray(Wv)], axis=0
    ).astype(np.float32)
    bfull = np.concatenate(
        [np.asarray(bq), np.asarray(bk), np.asarray(bv)], axis=0
    ).astype(np.float32)
    Wo = np.asarray(Wo, dtype=np.float32)
    q_norm_w = np.asarray(q_norm_w, dtype=np.float32)
    k_norm_w = np.asarray(k_norm_w, dtype=np.float32)

    xT_b = [np.ascontiguousarray(x[b].T).astype(bf) for b in range(B)]
    IDEN = np.eye(128, dtype=np.float32).astype(bf)
    wrep = np.stack(
        [np.tile(q_norm_w, 2), np.tile(k_norm_w, 2)], axis=1
    ).astype(np.float32)

    in_maps = []
    for core in range(8):
        b = core // 4
        hg = core % 4
        heads = [hg * 4 + i for i in range(G)]
        q_rows = np.concatenate([Wfull[192 * h : 192 * h + 64] for h in heads], axis=0)
        k_rows = np.concatenate(
            [Wfull[192 * h + 64 : 192 * h + 128] for h in heads], axis=0
        )
        v_rows = np.concatenate(
            [Wfull[192 * h + 128 : 192 * h + 192] for h in heads], axis=0
        )
        W_shard = np.concatenate([q_rows, k_rows, v_rows], axis=0)  # [768, 1024]
        bq_rows = np.concatenate([bfull[192 * h : 192 * h + 64] for h in heads])
        bk_rows = np.concatenate([bfull[192 * h + 64 : 192 * h + 128] for h in heads])
        bv_rows = np.concatenate([bfull[192 * h + 128 : 192 * h + 192] for h in heads])
        b_shard = np.concatenate([bq_rows, bk_rows, bv_rows])  # [768]
        cols = np.concatenate([np.arange(64 * h, 64 * h + 64) for h in heads])
        WoT_shard = np.ascontiguousarray(Wo[:, cols].T)  # [256, 1024]

        in_maps.append(
            {
                "xT": xT_b[b],
                "wqkvT": np.ascontiguousarray(W_shard.T).astype(bf),
                "bqkv": np.ascontiguousarray(b_shard.reshape(OC, 128).T).astype(
                    np.float32
                ),
                "wrep": wrep,
                "iden": IDEN,
                "woT": WoT_shard.reshape(2, 128, C).astype(bf),
            }
        )
    return in_maps


def kernel(**inputs):
    if "nc" not in _CACHE:
        _CACHE["nc"] = _build_nc()
    nc = _CACHE["nc"]
    in_maps = _prep_inputs(**inputs)
    res = run_bass_kernel_spmd(nc, in_maps, list(range(8)))
    bo = np.asarray(inputs["bo"], dtype=np.float32)
    y = np.zeros((B, N, C), dtype=np.float32)
    for core in range(8):
        y[core // 4] += res.results[core]["y"]
    y += bo[None, None, :]
    return y


# revision 10
# speedup vs baseline: 1.0112x; 1.0084x over previous
"""Trainium2 Bass kernel for nn_Attention_13537736917778.

Full inputs -> full output. Sharding: 8 cores = 2 (batch) x 4 (head groups of 4).
Per-core: channel-major flash attention (S^T layout, keys on partitions).
Softmax denominators are produced replicated across 64 PSUM rows by ones-columns
in the PV stationary operand; normalization and RMS-norm reciprocals both run as
ACT ln/exp pairs (exp table set only), so no DVE reciprocal and no PE broadcast
matmuls. Out-projection partial sums are reduced on host.
"""
import sys
import numpy as np

sys.path.insert(0, "/opt/trn_rl_repo")

import ml_dtypes
import concourse.bass as bass
import concourse.mybir as mybir
from concourse import tile
from concourse.bass_utils import run_bass_kernel_spmd
from contextlib import ExitStack

bf16 = mybir.dt.bfloat16
f32 = mybir.dt.float32

B, N, C = 2, 2048, 1024
H, D = 16, 64
G = 4              # heads per core
NT = N             # tokens per core (one batch)
FT = 512
TI = NT // FT      # 4 i-tiles
KC = C // 128      # 8 input-channel chunks
JC = NT // 128     # 16 key chunks
OC = 3 * G * D // 128   # 6 qkv output chunks
EPS = 1e-6
SCALE = 1.0 / 8.0  # 1/sqrt(64)

_CACHE = {}


def _split_waits(nc, limit=1):
    """walrus CTRL has one hw wait slot; split multi-wait instructions into
    NOP chains carrying the extra waits."""
    counter = 0
    for fn in nc.m.functions:
        for bb in fn.blocks:
            new_insts = []
            for inst in bb.instructions:
                si = inst.sync_info
                if si is not None and si.on_wait and len(si.on_wait) > limit:
                    waits = list(si.on_wait)
                    head, tail = waits[:-limit], waits[-limit:]
                    for i in range(0, len(head), limit):
                        nop = mybir.InstNoOp(
                            name=f"I-waitsplit-{counter}", ins=[], outs=[]
                        )
                        counter += 1
                        nop.engine = inst.engine
                        nop.sync_info = mybir.SyncInfo(
                            on_wait=head[i : i + limit], on_update=[]
                        )
                        new_insts.append(nop)
                    inst.sync_info = mybir.SyncInfo(
                        on_wait=tail, on_update=list(si.on_update)
                    )
                new_insts.append(inst)
            bb.instructions[:] = new_insts
    return counter


def _build_nc():
    nc = bass.Bass()
    xT = nc.declare_dram_parameter("xT", [C, NT], bf16, isOutput=False)
    wqkvT = nc.declare_dram_parameter("wqkvT", [C, OC * 128], bf16, isOutput=False)
    bqkv = nc.declare_dram_parameter("bqkv", [128, OC], f32, isOutput=False)
    wrep = nc.declare_dram_parameter("wrep", [128, 2], f32, isOutput=False)
    iden = nc.declare_dram_parameter("iden", [128, 128], bf16, isOutput=False)
    woT = nc.declare_dram_parameter("woT", [2, 128, C], bf16, isOutput=False)
    y = nc.declare_dram_parameter("y", [NT, C], f32, isOutput=True)

    xT_r = xT.rearrange("(kc p) n -> kc p n", p=128)
    wqkvT_r = wqkvT.rearrange("(kc p) o -> kc p o", p=128)

    Exp = mybir.ActivationFunctionType.Exp
    Ln = mybir.ActivationFunctionType.Ln
    MUL = mybir.AluOpType.mult

    with tile.TileContext(nc) as tc:
        with ExitStack() as ctx:
            perm = ctx.enter_context(tc.tile_pool(name="perm", bufs=1))
            # ---- persistent tiles ----
            iden_sb = perm.tile([128, 128], bf16, name="iden_sb", tag="iden_sb")
            nc.sync.dma_start(out=iden_sb[:], in_=iden[:])
            bias_sb = perm.tile([128, OC + 1], f32, name="bias_sb", tag="bias_sb")
            nc.sync.dma_start(out=bias_sb[:, 0:OC], in_=bqkv[:])
            nc.vector.memset(bias_sb[:, OC : OC + 1], EPS)
            wrep_sb = perm.tile([128, 2], f32, name="wrep_sb", tag="wrep_sb")
            nc.sync.dma_start(out=wrep_sb[:], in_=wrep[:])
            ones_mask = perm.tile([128, 128], bf16, name="ones_mask", tag="ones_mask")
            nc.vector.memset(ones_mask[:], 0.0)
            nc.vector.memset(ones_mask[0:64, 0:64], 1.0)
            nc.vector.memset(ones_mask[64:128, 64:128], 1.0)

            w_all = perm.tile([128, KC, OC * 128], bf16, name="w_all", tag="w_all")
            nc.sync.dma_start(
                out=w_all[:], in_=wqkvT.rearrange("(kc p) o -> p kc o", p=128)
            )
            woT_sb = []
            for oo in range(2):
                t = perm.tile([128, C], bf16, name=f"woT_sb{oo}", tag=f"woT_sb{oo}")
                nc.sync.dma_start(out=t[:], in_=woT[oo])
                woT_sb.append(t)

            qk_sb = [
                perm.tile([128, NT], f32, name=f"qk_sb{i}", tag=f"qk_sb{i}")
                for i in range(4)
            ]
            v16_sb = [
                perm.tile([128, NT], bf16, name=f"v16_sb{i}", tag=f"v16_sb{i}")
                for i in range(2)
            ]
            qhat = [
                perm.tile([128, NT], bf16, name=f"qhat{i}", tag=f"qhat{i}")
                for i in range(2)
            ]
            khat = [
                perm.tile([128, NT], bf16, name=f"khat{i}", tag=f"khat{i}")
                for i in range(2)
            ]
            # per jc: [A-V(64) | A-ones(64) | B-V(64) | B-ones(64)]
            vtok = [
                perm.tile([128, JC, 256], bf16, name=f"vtok{i}", tag=f"vtok{i}")
                for i in range(2)
            ]
            onT = [
                perm.tile([128, NT], bf16, name=f"onT{i}", tag=f"onT{i}")
                for i in range(2)
            ]
            for c in range(2):
                nc.vector.memset(vtok[c][:, :, 64:128], 1.0)
                nc.vector.memset(vtok[c][:, :, 192:256], 1.0)

            # ---- pools (whole kernel) ----
            sqpool = ctx.enter_context(tc.tile_pool(name="sqpool", bufs=2))
            t1pool = ctx.enter_context(tc.tile_pool(name="t1pool", bufs=1))
            rrpool = ctx.enter_context(tc.tile_pool(name="rrpool", bufs=2))
            ptpool = ctx.enter_context(tc.tile_pool(name="ptpool", bufs=3))
            dnpool = ctx.enter_context(tc.tile_pool(name="dnpool", bufs=1))
            ypool = ctx.enter_context(tc.tile_pool(name="ypool", bufs=2))
            # PSUM: tag "s4" = 2 slots x [128,1536] (6 banks), tag "pv" = 1 slot x
            # [128,1024] (2 banks)
            spool = ctx.enter_context(tc.tile_pool(name="spool", bufs=2, space="PSUM"))
            pvpool = ctx.enter_context(
                tc.tile_pool(name="pvpool", bufs=1, space="PSUM")
            )

            cnt = [0]
            xt_all = perm.tile([128, KC, NT], bf16, name="xt_all", tag="xt_all")

            def emit_x_dma(kc):
                nc.sync.dma_start(out=xt_all[:, kc, :], in_=xT_r[kc])

            def emit_qkv_group(it, oc):
                isl = slice(it * FT, (it + 1) * FT)
                cnt[0] += 1
                ps = spool.tile(
                    [128, 3 * FT], f32, name=f"qp{cnt[0]}", tag="s4"
                )
                for kc in range(KC):
                    nc.tensor.matmul(
                        ps[:, 0:FT],
                        lhsT=w_all[:, kc, oc * 128 : (oc + 1) * 128],
                        rhs=xt_all[:, kc, isl],
                        start=(kc == 0),
                        stop=(kc == KC - 1),
                    )
                dst = qk_sb[oc] if oc < 4 else v16_sb[oc - 4]
                nc.vector.tensor_scalar_add(
                    dst[:, isl], ps[:, 0:FT], bias_sb[:, oc : oc + 1]
                )

            def emit_vtrans(c, jc):
                cnt[0] += 1
                pt = spool.tile([128, 128], bf16, name=f"vt{cnt[0]}", tag="s4")
                nc.tensor.transpose(
                    pt[:], v16_sb[c][:, jc * 128 : (jc + 1) * 128], iden_sb[:]
                )
                nc.vector.tensor_copy(vtok[c][:, jc, 0:64], pt[:, 0:64])
                nc.vector.tensor_copy(vtok[c][:, jc, 128:192], pt[:, 64:128])

            def emit_rms(qc, qk_i):
                src_t = qk_sb[qc] if qk_i == 0 else qk_sb[2 + qc]
                dst = qhat[qc] if qk_i == 0 else khat[qc]
                sq = sqpool.tile([128, NT], bf16, name=f"sq{qc}_{qk_i}", tag="sq")
                nc.vector.tensor_mul(sq[:], src_t[:], src_t[:])
                rr = rrpool.tile([128, NT], f32, name=f"rr{qc}_{qk_i}", tag="rr")
                for half in range(2):
                    hs = slice(half * 2 * FT, (half + 1) * 2 * FT)
                    cnt[0] += 1
                    ms = pvpool.tile(
                        [128, 2 * FT], f32, name=f"ms{cnt[0]}", tag="pv"
                    )
                    for t in range(2):
                        tsl = slice(t * FT, (t + 1) * FT)
                        gsl = slice((half * 2 + t) * FT, (half * 2 + t + 1) * FT)
                        nc.tensor.matmul(
                            ms[:, tsl],
                            lhsT=ones_mask[:],
                            rhs=sq[:, gsl],
                            start=True,
                            stop=True,
                        )
                    t1 = t1pool.tile(
                        [128, 2 * FT], f32, name=f"t1_{qc}_{qk_i}_{half}", tag="t1"
                    )
                    nc.scalar.activation(
                        t1[:], ms[:], Ln, scale=1.0 / D, bias=bias_sb[:, OC : OC + 1]
                    )
                    nc.scalar.activation(rr[:, hs], t1[:], Exp, scale=-0.5)
                nc.vector.scalar_tensor_tensor(
                    dst[:], src_t[:], wrep_sb[:, qk_i : qk_i + 1], rr[:], MUL, MUL
                )

            def emit_attention(qc, it, sprinkle=None):
                # sprinkle: dict s -> list of thunks to emit after step s
                isl = slice(it * FT, (it + 1) * FT)
                pv2 = pvpool.tile(
                    [128, 2 * FT], f32, name=f"pv2_{qc}_{it}", tag="pv"
                )
                NSEQ = 2 * JC
                PER = 3
                s3 = None
                for s in range(NSEQ):
                    head = s % 2
                    jc = s // 2
                    jsl = slice(jc * 128, (jc + 1) * 128)
                    slot = s % PER
                    if slot == 0:
                        s3 = spool.tile(
                            [128, PER * FT], f32, name=f"s3_{qc}_{it}_{s}", tag="s4"
                        )
                    ssl = slice(slot * FT, (slot + 1) * FT)
                    hsl = slice(head * 64, (head + 1) * 64)
                    nc.tensor.matmul(
                        s3[:, ssl],
                        lhsT=khat[qc][hsl, jsl],
                        rhs=qhat[qc][hsl, isl],
                        start=True,
                        stop=True,
                    )
                    if slot == PER - 1 or s == NSEQ - 1:
                        nfull = slot + 1
                        pt3 = ptpool.tile(
                            [128, PER * FT], bf16, name=f"pt{qc}_{it}_{s}", tag="pt"
                        )
                        nc.scalar.activation(
                            pt3[:, 0 : nfull * FT],
                            s3[:, 0 : nfull * FT],
                            Exp,
                            scale=SCALE,
                        )
                        for sb in range(s - nfull + 1, s + 1):
                            hb = sb % 2
                            jb = sb // 2
                            bsl = slice((sb % PER) * FT, (sb % PER + 1) * FT)
                            nc.tensor.matmul(
                                pv2[:, hb * FT : (hb + 1) * FT],
                                lhsT=vtok[qc][:, jb, hb * 128 : (hb + 1) * 128],
                                rhs=pt3[:, bsl],
                                start=(sb == hb),
                                stop=(sb >= NSEQ - 2),
                            )
                    if sprinkle and s in sprinkle:
                        for thunk in sprinkle[s]:
                            thunk()
                # normalize: O = PV * exp(-ln(denom)); denom replicated rows 64:128
                td = dnpool.tile([64, 2 * FT], f32, name=f"td{qc}_{it}", tag="td")
                nc.scalar.activation(td[:], pv2[64:128, :], Ln)
                bcr = dnpool.tile([64, 2 * FT], f32, name=f"bcr{qc}_{it}", tag="bcr")
                nc.scalar.activation(bcr[:], td[:], Exp, scale=-1.0)
                nc.vector.tensor_mul(onT[qc][0:64, isl], pv2[0:64, 0:FT], bcr[:, 0:FT])
                nc.vector.tensor_mul(
                    onT[qc][64:128, isl], pv2[0:64, FT : 2 * FT], bcr[:, FT : 2 * FT]
                )

            def emit_outproj(ic, tagname):
                csl = slice(ic * 128, (ic + 1) * 128)
                shape = [128, 3 * FT] if tagname == "s4" else [128, 2 * FT]
                pool = spool if tagname == "s4" else pvpool
                p01 = pool.tile(shape, f32, name=f"p01_{ic}", tag=tagname)
                for oo in range(2):
                    nc.tensor.matmul(
                        p01[:, 0:FT],
                        lhsT=onT[oo][:, csl],
                        rhs=woT_sb[oo][:, 0:FT],
                        start=(oo == 0),
                        stop=(oo == 1),
                    )
                    nc.tensor.matmul(
                        p01[:, FT : 2 * FT],
                        lhsT=onT[oo][:, csl],
                        rhs=woT_sb[oo][:, FT : 2 * FT],
                        start=(oo == 0),
                        stop=(oo == 1),
                    )
                yt = ypool.tile([128, C], f32, name=f"yt{ic}", tag="yt")
                nc.vector.tensor_copy(yt[:], p01[:, 0 : 2 * FT])
                nc.sync.dma_start(out=y[csl, :], in_=yt[:])

            # ---------------- emission schedule ----------------
            for kc in range(KC):
                emit_x_dma(kc)
            # pair-0 projections with early rms so ACT starts ASAP
            for it in range(TI):
                emit_qkv_group(it, 0)
            emit_rms(0, 0)
            for it in range(TI):
                emit_qkv_group(it, 2)
            emit_rms(0, 1)
            for it in range(TI):
                emit_qkv_group(it, 4)
                for jc in range(it * 4, it * 4 + 4):
                    emit_vtrans(0, jc)
            # attention for pair 0, with pair-1 projections sprinkled in
            for it in range(TI):
                spr = {
                    7: [lambda it=it: emit_qkv_group(it, 1)],
                    15: [lambda it=it: emit_qkv_group(it, 3)],
                    23: [lambda it=it: emit_qkv_group(it, 5)]
                    + (
                        [
                            lambda it=it: [
                                emit_vtrans(1, jc)
                                for jc in range((it - 1) * 4, it * 4)
                            ]
                        ]
                        if it > 0
                        else []
                    ),
                }
                emit_attention(0, it, sprinkle=spr)
            for jc in range(12, 16):
                emit_vtrans(1, jc)
            emit_rms(1, 0)
            emit_rms(1, 1)
            # attention for pair 1, with out-projection of prior i-tiles sprinkled in
            for it in range(TI):
                if it == 0:
                    spr = None
                else:
                    base = (it - 1) * 4
                    spr = {
                        7: [lambda ic=base: emit_outproj(ic, "s4")],
                        15: [lambda ic=base + 1: emit_outproj(ic, "s4")],
                        23: [
                            lambda ic=base + 2: emit_outproj(ic, "s4"),
                            lambda ic=base + 3: emit_outproj(ic, "s4"),
                        ],
                    }
                emit_attention(1, it, sprinkle=spr)
            for ic in range(12, 16):
                emit_outproj(ic, ("s4", "s4", "pv", "s4")[ic % 4])

    _split_waits(nc, limit=1)
    return nc


def _prep_inputs(x, Wq, bq, Wk, bk, Wv, bv, q_norm_w, k_norm_w, Wo, bo):
    bf = ml_dtypes.bfloat16
    x = np.asarray(x, dtype=np.float32)
    Wfull = np.concatenate(
        [np.asarray(Wq), np.asarray(Wk), np.asarray(Wv)], axis=0
    ).astype(np.float32)
    bfull = np.concatenate(
        [np.asarray(bq), np.asarray(bk), np.asarray(bv)], axis=0
    ).astype(np.float32)
    Wo = np.asarray(Wo, dtype=np.float32)
    q_norm_w = np.asarray(q_norm_w, dtype=np.float32)
    k_norm_w = np.asarray(k_norm_w, dtype=np.float32)

    xT_b = [np.ascontiguousarray(x[b].T).astype(bf) for b in range(B)]
    IDEN = np.eye(128, dtype=np.float32).astype(bf)
    wrep = np.stack(
        [np.tile(q_norm_w, 2), np.tile(k_norm_w, 2)], axis=1
    ).astype(np.float32)

    in_maps = []
    for core in range(8):
        b = core // 4
        hg = core % 4
        heads = [hg * 4 + i for i in range(G)]
        q_rows = np.concatenate([Wfull[192 * h : 192 * h + 64] for h in heads], axis=0)
        k_rows = np.concatenate(
            [Wfull[192 * h + 64 : 192 * h + 128] for h in heads], axis=0
        )
        v_rows = np.concatenate(
            [Wfull[192 * h + 128 : 192 * h + 192] for h in heads], axis=0
        )
        W_shard = np.concatenate([q_rows, k_rows, v_rows], axis=0)  # [768, 1024]
        bq_rows = np.concatenate([bfull[192 * h : 192 * h + 64] for h in heads])
        bk_rows = np.concatenate([bfull[192 * h + 64 : 192 * h + 128] for h in heads])
        bv_rows = np.concatenate([bfull[192 * h + 128 : 192 * h + 192] for h in heads])
        b_shard = np.concatenate([bq_rows, bk_rows, bv_rows])  # [768]
        cols = np.concatenate([np.arange(64 * h, 64 * h + 64) for h in heads])
        WoT_shard = np.ascontiguousarray(Wo[:, cols].T)  # [256, 1024]

        in_maps.append(
            {
                "xT": xT_b[b],
                "wqkvT": np.ascontiguousarray(W_shard.T).astype(bf),
                "bqkv": np.ascontiguousarray(b_shard.reshape(OC, 128).T).astype(
                    np.float32
                ),
                "wrep": wrep,
                "iden": IDEN,
                "woT": WoT_shard.reshape(2, 128, C).astype(bf),
            }
        )
    return in_maps


def kernel(**inputs):
    if "nc" not in _CACHE:
        _CACHE["nc"] = _build_nc()
    nc = _CACHE["nc"]
    in_maps = _prep_inputs(**inputs)
    res = run_bass_kernel_spmd(nc, in_maps, list(range(8)))
    bo = np.asarray(inputs["bo"], dtype=np.float32)
    y = np.zeros((B, N, C), dtype=np.float32)
    for core in range(8):
        y[core // 4] += res.results[core]["y"]
    y += bo[None, None, :]
    return y
